# revision 3
# baseline (speedup 1.0000x reference)
"""Trainium2 Bass kernel for batched linear attention (nn_Linear_Attention).

Per (b,h) pair (64 total, 8 per NeuronCore):
    sigma(x) = elu(x)+1 = min(exp(x),1) + relu(x)
    A  = (sigma(q) @ M) / (sigma(q) @ Z)
    Mn = b*M + sigma(k)^T @ (v - (sigma(k) @ M)/(sigma(k) @ Z))
    Zn = b*Z + sum_n sigma(k)
Reassociation avoids materializing sigma(k)^T:
    Mn = b*M + sigma_k^T @ [v | 1] - (sigma_k^T diag(1/den_k) sigma_k) @ M
sigma(q)^T is produced via PE transposes.  Matmul operands are fp16
(fp32 PSUM accumulation); outputs are fp32.
"""

import numpy as np

B, H, SEQ, D, V = 4, 16, 4096, 128, 128
N_CORES = 8
PAIRS_PER_CORE = (B * H) // N_CORES  # 8
NT = SEQ // 128  # 32 row-tiles

_compiled_cache = {}


def _build_nc(b_scalar: float):
    import concourse.bass as bass
    import concourse.bacc as bacc
    import concourse.tile as tile
    from concourse import mybir
    from concourse.masks import make_identity

    f32 = mybir.dt.float32
    f16 = mybir.dt.float16
    Alu = mybir.AluOpType
    Act = mybir.ActivationFunctionType

    nc = bacc.Bacc("TRN2", target_bir_lowering=False, debug=False,
                   num_devices=N_CORES)

    P = PAIRS_PER_CORE
    q_in = nc.dram_tensor("q", [P, SEQ, D], f32, kind="ExternalInput").ap()
    k_in = nc.dram_tensor("k", [P, SEQ, D], f32, kind="ExternalInput").ap()
    v_in = nc.dram_tensor("v", [P, SEQ, V], f32, kind="ExternalInput").ap()
    M_in = nc.dram_tensor("M", [P, D, V], f32, kind="ExternalInput").ap()
    Z_in = nc.dram_tensor("Z", [P, D, 1], f32, kind="ExternalInput").ap()
    A_out = nc.dram_tensor("A", [P, SEQ, V], f32, kind="ExternalOutput").ap()
    Mn_out = nc.dram_tensor("Mn", [P, D, V], f32, kind="ExternalOutput").ap()
    Zn_out = nc.dram_tensor("Zn", [P, D, 1], f32, kind="ExternalOutput").ap()

    with tile.TileContext(nc) as tc:
        with tc.tile_pool(name="consts", bufs=1) as consts, \
             tc.tile_pool(name="big", bufs=2) as big, \
             tc.tile_pool(name="small", bufs=2) as small, \
             tc.tile_pool(name="wpool", bufs=3) as wpool, \
             tc.tile_pool(name="apool", bufs=3) as apool, \
             tc.tile_pool(name="ps_s1", bufs=1, space="PSUM") as ps_s1_pool, \
             tc.tile_pool(name="ps_gt", bufs=1, space="PSUM") as ps_gt_pool, \
             tc.tile_pool(name="ps_qt", bufs=2, space="PSUM") as ps_qt_pool, \
             tc.tile_pool(name="ps_a", bufs=2, space="PSUM") as ps_a_pool, \
             tc.tile_pool(name="ps_dq", bufs=1, space="PSUM") as ps_dq_pool, \
             tc.tile_pool(name="ps_s2", bufs=1, space="PSUM") as ps_s2_pool:

            ident = consts.tile([128, 128], f16)
            make_identity(nc, ident)

            for p in range(PAIRS_PER_CORE):
                # ---------- per-pair constants ----------
                Mz16 = small.tile([128, V + 1], f16, tag="Mz16")
                nc.gpsimd.dma_start(out=Mz16[:, 0:V], in_=M_in[p])
                nc.gpsimd.dma_start(out=Mz16[:, V:V + 1], in_=Z_in[p])
                M_sb = small.tile([128, V], f32, tag="M_sb")
                nc.sync.dma_start(out=M_sb, in_=M_in[p])
                Z_sb = small.tile([128, 1], f32, tag="Z_sb")
                nc.sync.dma_start(out=Z_sb, in_=Z_in[p])
                # Z replicated across partitions (free dim indexes d), fp16
                zrep = small.tile([128, D], f16, tag="zrep")
                z_flat = Z_in[p].rearrange("d 1 -> (d 1)")
                z_bcast = bass.AP(
                    tensor=z_flat.tensor, offset=z_flat.offset,
                    ap=[[0, 128]] + z_flat.ap,
                )
                nc.gpsimd.dma_start(out=zrep, in_=z_bcast)

                # ---------- k path ----------
                kb = big.tile([128, NT, D], f16, tag="kb")
                nc.gpsimd.dma_start(
                    out=kb, in_=k_in[p].rearrange("(t pp) d -> pp t d", pp=128))
                vb = big.tile([128, NT, V + 1], f16, tag="vb")
                nc.vector.memset(vb[:, :, V:V + 1], 1.0)
                nc.gpsimd.dma_start(
                    out=vb[:, :, 0:V],
                    in_=v_in[p].rearrange("(t pp) d -> pp t d", pp=128))

                sig_k = big.tile([128, NT, D], f16, tag="sig_k")
                e_k = big.tile([128, NT, D], f16, tag="e_k")
                nc.scalar.activation(out=sig_k, in_=kb, func=Act.Relu)
                nc.scalar.activation(out=e_k, in_=kb, func=Act.Exp)
                # sig_k = min(e_k, 1) + relu(kb)   (in-place add into sig_k)
                nc.vector.scalar_tensor_tensor(
                    out=sig_k, in0=e_k, scalar=1.0, in1=sig_k,
                    op0=Alu.min, op1=Alu.add)

                # den_k[n] = sum_d sig_k[n,d] * Z[d]
                dens_k = small.tile([128, NT], f32, tag="dens_k")
                junk = small.tile([128, D], f16, tag="junk")
                for t in range(NT):
                    nc.vector.scalar_tensor_tensor(
                        out=junk, in0=sig_k[:, t], scalar=0.0, in1=zrep,
                        op0=Alu.add, op1=Alu.mult,
                        accum_out=dens_k[:, t:t + 1])
                rk = small.tile([128, NT], f32, tag="rk")
                nc.vector.reciprocal(out=rk, in_=dens_k)

                ps_s1 = ps_s1_pool.tile([128, V + 1], f32)
                ps_gt = ps_gt_pool.tile([128, D], f32)
                # Gt = sum_t w_t^T @ sig_k_t
                for t in range(NT):
                    w_t = wpool.tile([128, D], f16, tag="w")
                    nc.vector.tensor_scalar_mul(
                        out=w_t, in0=sig_k[:, t], scalar1=rk[:, t:t + 1])
                    nc.tensor.matmul(ps_gt, w_t, sig_k[:, t],
                                     start=(t == 0), stop=(t == NT - 1))
                gt16 = small.tile([128, D], f16, tag="gt16")
                nc.vector.tensor_scalar_mul(out=gt16, in0=ps_gt, scalar1=-1.0)
                # S1 = sum_t sig_k_t^T @ [v_t | 1]
                for t in range(NT):
                    nc.tensor.matmul(ps_s1, sig_k[:, t], vb[:, t],
                                     start=(t == 0), stop=(t == NT - 1))
                # -S2 = gt16^T @ M in its own bank/group
                ps_s2 = ps_s2_pool.tile([128, V], f32)
                nc.tensor.matmul(ps_s2, gt16, Mz16[:, 0:V],
                                 start=True, stop=True)

                mn_sb = small.tile([128, V], f32, tag="mn_sb")
                nc.vector.scalar_tensor_tensor(
                    out=mn_sb, in0=M_sb, scalar=b_scalar, in1=ps_s1[:, 0:V],
                    op0=Alu.mult, op1=Alu.add)
                nc.vector.tensor_add(out=mn_sb, in0=mn_sb, in1=ps_s2)
                zn_sb = small.tile([128, 1], f32, tag="zn_sb")
                nc.vector.scalar_tensor_tensor(
                    out=zn_sb, in0=Z_sb, scalar=b_scalar,
                    in1=ps_s1[:, V:V + 1], op0=Alu.mult, op1=Alu.add)
                nc.sync.dma_start(out=Mn_out[p], in_=mn_sb)
                nc.sync.dma_start(out=Zn_out[p], in_=zn_sb)

                # ---------- q path ----------
                qb = big.tile([128, NT, D], f16, tag="qb")
                nc.gpsimd.dma_start(
                    out=qb, in_=q_in[p].rearrange("(t pp) d -> pp t d", pp=128))
                sig_q = big.tile([128, NT, D], f16, tag="sig_q")
                e_q = big.tile([128, NT, D], f16, tag="e_q")
                nc.scalar.activation(out=sig_q, in_=qb, func=Act.Relu)
                nc.scalar.activation(out=e_q, in_=qb, func=Act.Exp)
                nc.vector.scalar_tensor_tensor(
                    out=sig_q, in0=e_q, scalar=1.0, in1=sig_q,
                    op0=Alu.min, op1=Alu.add)

                # pass 1: PE-transpose sigma_q tiles; denominator matmuls
                sig_qt = big.tile([128, NT, D], f16, tag="sig_qt")
                ps_dq = ps_dq_pool.tile([128, NT], f32)
                for t in range(NT):
                    ps_qt = ps_qt_pool.tile([128, 128], f16)
                    nc.tensor.transpose(ps_qt, sig_q[:, t], ident)
                    if t % 2 == 0:
                        nc.vector.tensor_copy(out=sig_qt[:, t], in_=ps_qt)
                    else:
                        nc.scalar.copy(out=sig_qt[:, t], in_=ps_qt)
                    nc.tensor.matmul(ps_dq[:, t:t + 1], sig_qt[:, t],
                                     Mz16[:, V:V + 1], start=True, stop=True)
                rq = small.tile([128, NT], f32, tag="rq")
                nc.vector.reciprocal(out=rq, in_=ps_dq)

                # pass 2: A matmuls + fused divide (PSUM->SBUF)
                for c in range(NT // 4):
                    a_sb = apool.tile([128, 4, V], f32, tag="a_sb")
                    for j in range(4):
                        t = 4 * c + j
                        ps_a = ps_a_pool.tile([128, V], f32)
                        nc.tensor.matmul(ps_a, sig_qt[:, t], Mz16[:, 0:V],
                                         start=True, stop=True)
                        if t % 2 == 0:
                            nc.scalar.activation(
                                out=a_sb[:, j], in_=ps_a, func=Act.Copy,
                                scale=rq[:, t:t + 1])
                        else:
                            nc.vector.tensor_scalar_mul(
                                out=a_sb[:, j], in0=ps_a,
                                scalar1=rq[:, t:t + 1])
                    nc.sync.dma_start(
                        out=A_out[p].rearrange(
                            "(c j pp) d -> pp c j d", pp=128, j=4)[:, c],
                        in_=a_sb)

    nc.compile()
    return nc


def kernel(q, k, v, M, Z, beta):
    q = np.ascontiguousarray(q, dtype=np.float32)
    k = np.ascontiguousarray(k, dtype=np.float32)
    v = np.ascontiguousarray(v, dtype=np.float32)
    M = np.ascontiguousarray(M, dtype=np.float32)
    Z = np.ascontiguousarray(Z, dtype=np.float32)
    beta = np.asarray(beta, dtype=np.float32)

    b_scalar = float(np.clip(1.0 / (1.0 + np.exp(-beta[0])), 0.9, 0.999))

    key = round(b_scalar, 6)
    if key not in _compiled_cache:
        _compiled_cache[key] = _build_nc(b_scalar)
    nc = _compiled_cache[key]

    BH = B * H
    P = PAIRS_PER_CORE
    qr = q.reshape(BH, SEQ, D)
    kr = k.reshape(BH, SEQ, D)
    vr = v.reshape(BH, SEQ, V)
    Mr = M.reshape(BH, D, V)
    Zr = Z.reshape(BH, D, 1)

    in_maps = []
    for c in range(N_CORES):
        s = slice(c * P, (c + 1) * P)
        in_maps.append({
            "q": qr[s], "k": kr[s], "v": vr[s], "M": Mr[s], "Z": Zr[s],
        })

    from concourse.bass_utils import run_bass_kernel_spmd
    res = run_bass_kernel_spmd(nc, in_maps, list(range(N_CORES)))

    A = np.concatenate([res.results[c]["A"] for c in range(N_CORES)], axis=0)
    Mn = np.concatenate([res.results[c]["Mn"] for c in range(N_CORES)], axis=0)
    Zn = np.concatenate([res.results[c]["Zn"] for c in range(N_CORES)], axis=0)
    return (
        A.reshape(B, H, SEQ, V),
        Mn.reshape(B, H, D, V),
        Zn.reshape(B, H, D, 1),
    )
